# revision 15
# baseline (speedup 1.0000x reference)
"""Trainium2 Bass kernel for nn_AttLayer (sliding-block attention encoder layer).

Sharding: 8 cores = 4 batches x 2 sequence halves (4096 frames each).
Each core gets its x1 slice with a 256-frame halo on both sides (zero-padded at
sequence edges), computes q/k/v projections, 8 blocks of windowed attention
(block 512, window 1024), relu + output projection locally. No collectives.

Device layout choices:
  - q, k stored [c=256(2 ptiles), Lext=4608] in SBUF.
  - v stored TRANSPOSED [Lext(36 ptiles), c3=256]  (computed directly as
    x^T @ Wv^T so no on-chip transpose is ever needed).
  - energy computed transposed: eT[k, q] = sum_c k[c,k] q[c,q]  -> the softmax
    log-mask bias lands on the partition dim, a perfect fit for the ACT
    engine's per-partition bias operand:  P = Exp(eT/16 + bias).
  - no max-subtraction in softmax (energies are O(10), exp is safe in fp32).
  - row sums s[q] via ones-vector matmul (M=1) accumulated with AV.
  - 1/s broadcast across partitions via a K=1 ones matmul.
  - biases: bq/bk added at PSUM evacuation (per-partition); bv folded into the
    ReLU's per-partition bias AFTER normalization (exact: relu(x/s + bv));
    bo added at final evacuation.
"""

import numpy as np

# problem constants (self-contained; must match the harness reference)
B, CIN, L = 4, 512, 8192
C, VD = 256, 512
BL, HALF = 512, 256
NCORES = 8
LCH = L // 2            # 4096 frames per core
LEXT = LCH + 2 * HALF   # 4608 with halo
NBLK = LCH // BL        # 8 local blocks
WS = BL + 2 * HALF      # 1024 window
NKT = WS // 128         # 8 k-tiles per window
NCH = LEXT // BL        # 9 x chunks
NVT = LEXT // 128       # 36 v^T partition tiles

MM_DTYPE = "float32r"   # matmul compute dtype: "float32r" (fast) or "float32"

_NC_CACHE = {}


def _build_nc():
    import concourse.bacc as bacc
    import concourse.mybir as mybir
    import concourse.tile as tile
    from contextlib import ExitStack

    f32 = mybir.dt.float32
    rdt = getattr(mybir.dt, MM_DTYPE)
    AF = mybir.ActivationFunctionType

    nc = bacc.Bacc("TRN2", target_bir_lowering=False, debug=False,
                   num_devices=NCORES)

    x_d = nc.dram_tensor("x", [CIN, LEXT], rdt, kind="ExternalInput").ap()
    wq_d = nc.dram_tensor("wq_t", [CIN, C], rdt, kind="ExternalInput").ap()
    wk_d = nc.dram_tensor("wk_t", [CIN, C], rdt, kind="ExternalInput").ap()
    wv_d = nc.dram_tensor("wv_t", [CIN, C], rdt, kind="ExternalInput").ap()
    wo_d = nc.dram_tensor("wo_t", [C, VD], rdt, kind="ExternalInput").ap()
    bq_d = nc.dram_tensor("bq", [C, 1], f32, kind="ExternalInput").ap()
    bk_d = nc.dram_tensor("bk", [C, 1], f32, kind="ExternalInput").ap()
    bv_d = nc.dram_tensor("bv", [C, 1], f32, kind="ExternalInput").ap()
    bo_d = nc.dram_tensor("bo", [VD, 1], f32, kind="ExternalInput").ap()
    ab_d = nc.dram_tensor("abias", [128, NBLK * NKT], f32,
                          kind="ExternalInput").ap()
    out_d = nc.dram_tensor("out", [VD, LCH], f32, kind="ExternalOutput").ap()

    x_r = x_d.rearrange("(r p) l -> p r l", p=128)      # [128, 4, 4608]
    wq_r = wq_d.rearrange("(r p) c -> p r c", p=128)    # [128, 4, 256]
    wk_r = wk_d.rearrange("(r p) c -> p r c", p=128)
    wv_r = wv_d.rearrange("(r p) c -> p r c", p=128)
    wo_r = wo_d.rearrange("(m p) v -> p m v", p=128)    # [128, 2, 512]
    bq_r = bq_d.rearrange("(m p) o -> p m o", p=128)    # [128, 2, 1]
    bk_r = bk_d.rearrange("(m p) o -> p m o", p=128)
    bv_r = bv_d.rearrange("(m p) o -> p m o", p=128)
    bo_r = bo_d.rearrange("(v p) o -> p v o", p=128)    # [128, 4, 1]
    out_r = out_d.rearrange("(v p) l -> p v l", p=128)  # [128, 4, 4096]

    with tile.TileContext(nc) as tc:
        with ExitStack() as ctx:
            ctx.enter_context(nc.allow_low_precision(
                reason="float32r matmul pipeline; fp32 PSUM accumulation"))
            sbc = ctx.enter_context(tc.tile_pool(name="sbc", bufs=1))  # constants
            sbp = ctx.enter_context(tc.tile_pool(name="sbp", bufs=1))  # persistent
            sbs = ctx.enter_context(tc.tile_pool(name="sbs", bufs=1))  # streaming
            ps = ctx.enter_context(tc.tile_pool(name="ps", bufs=1, space="PSUM"))

            dma = nc.sync.dma_start

            wq = sbc.tile([128, 4, C], rdt, tag="wq", name="wq")
            wk = sbc.tile([128, 4, C], rdt, tag="wk", name="wk")
            wv = sbc.tile([128, 4, C], rdt, tag="wv", name="wv")
            # chunk-0 x tile is hoisted so its DMAs interleave with wq's:
            # the Sync engine issues one DMA per ~650ns, so issue order is
            # the critical path to the first matmul.
            xt0 = sbs.tile([128, 4, BL], rdt, tag="x", bufs=3, name="xt0")
            xt1 = sbs.tile([128, 4, BL], rdt, tag="x", bufs=3, name="xt1")
            bq = sbc.tile([128, 2, 1], f32, tag="bq", name="bq")
            bk = sbc.tile([128, 2, 1], f32, tag="bk", name="bk")
            dma(out=wq[:, 0:1, :], in_=wq_r[:, 0:1, :])
            dma(out=xt0[:, 0:1, :], in_=x_r[:, 0:1, 0:BL])
            dma(out=bq[:], in_=bq_r)
            dma(out=bk[:], in_=bk_r)
            for r in range(1, 4):
                dma(out=wq[:, r:r + 1, :], in_=wq_r[:, r:r + 1, :])
                dma(out=xt0[:, r:r + 1, :], in_=x_r[:, r:r + 1, 0:BL])
            xt2 = sbs.tile([128, 4, BL], rdt, tag="x", bufs=3, name="xt2")
            for r in range(4):
                dma(out=wk[:, r:r + 1, :], in_=wk_r[:, r:r + 1, :])
                dma(out=wv[:, r:r + 1, :], in_=wv_r[:, r:r + 1, :])
                dma(out=xt1[:, r:r + 1, :], in_=x_r[:, r:r + 1, BL:2 * BL])
                dma(out=xt2[:, r:r + 1, :], in_=x_r[:, r:r + 1, 2 * BL:3 * BL])

            # PE warmup: ~30 dependency-free bf16 matmuls during the DMA head
            # so the HAM clock-gate reaches 8/8 before real work arrives.
            bf16 = mybir.dt.bfloat16
            wrm = sbc.tile([128, BL], bf16, tag="wrm", name="wrm")
            nc.vector.memset(wrm[:], 0.0)
            wps = ps.tile([128, BL], f32, tag="pp", bufs=2, name="wps")
            for i in range(20):
                nc.tensor.matmul(wps[:], wrm[:, 0:128], wrm[:], start=True,
                                 stop=True)

            def emit_late_consts():
                wo = sbc.tile([128, 2, VD], rdt, tag="wo", name="wo")
                dma(out=wo[:], in_=wo_r)
                bv = sbc.tile([128, 2, 1], f32, tag="bv", name="bv")
                dma(out=bv[:], in_=bv_r)
                bo = sbc.tile([128, 4, 1], f32, tag="bo", name="bo")
                dma(out=bo[:], in_=bo_r)
                ab = sbc.tile([128, NBLK * NKT], f32, tag="ab", name="ab")
                dma(out=ab[:], in_=ab_d)
                ones_f = sbc.tile([128, 1], f32, tag="ones_f", name="ones_f")
                nc.vector.memset(ones_f[:], 1.0)
                ones_k = sbc.tile([128, 1], rdt, tag="ones_k", name="ones_k")
                nc.vector.tensor_copy(ones_k[:], ones_f[:])
                return wo, bv, bo, ab, ones_k

            q_sb = [sbp.tile([128, LEXT], rdt, tag=f"qsb{i}", name=f"qsb{i}")
                    for i in range(2)]
            k_sb = [sbp.tile([128, LEXT], rdt, tag=f"ksb{i}", name=f"ksb{i}")
                    for i in range(2)]
            vts = [sbp.tile([128, C], rdt, tag=f"vt{i}", name=f"vt{i}")
                   for i in range(NVT)]

            def mm(out_ap, lhsT, rhs, start, stop):
                nc.tensor.matmul(out_ap, lhsT, rhs, start=start, stop=stop)

            # ---------------- projections (streamed over 9 x-chunks) --------
            def emit_chunk(c):
                if c == 0:
                    xt = xt0
                elif c == 1:
                    xt = xt1
                elif c == 2:
                    xt = xt2
                else:
                    xt = sbs.tile([128, 4, BL], rdt, tag="x", bufs=3,
                                  name=f"xt{c}")
                    for r in range(4):
                        dma(out=xt[:, r:r + 1, :],
                            in_=x_r[:, r:r + 1, c * BL:(c + 1) * BL])
                # q is only needed on extended cols [HALF, LEXT-HALF)
                qlo = max(c * BL, HALF) - c * BL
                qhi = min((c + 1) * BL, LEXT - HALF) - c * BL
                for o in range(2):
                    pq = ps.tile([128, BL], f32, tag="pp", bufs=2,
                                 name=f"pq{c}_{o}")
                    for r in range(4):
                        mm(pq[:, 0:qhi - qlo],
                           wq[:, r, o * 128:(o + 1) * 128],
                           xt[:, r, qlo:qhi], r == 0, r == 3)
                    nc.vector.tensor_scalar_add(
                        q_sb[o][:, c * BL + qlo:c * BL + qhi],
                        pq[:, 0:qhi - qlo], bq[:, o, :])
                for o in range(2):
                    pk = ps.tile([128, BL], f32, tag="pp", bufs=2,
                                 name=f"pk{c}_{o}")
                    for r in range(4):
                        mm(pk[:], wk[:, r, o * 128:(o + 1) * 128], xt[:, r, :],
                           r == 0, r == 3)
                    nc.vector.tensor_scalar_add(
                        k_sb[o][:, c * BL:(c + 1) * BL], pk[:], bk[:, o, :])
                for lt in range(4):
                    pv = ps.tile([128, C], f32, tag="pp", bufs=2,
                                 name=f"pv{c}_{lt}")
                    for r in range(4):
                        mm(pv[:], xt[:, r, lt * 128:(lt + 1) * 128],
                           wv[:, r, :], r == 0, r == 3)
                    nc.vector.tensor_copy(vts[c * 4 + lt][:], pv[:])

            emit_chunk(0)
            wo, bv, bo, ab, ones_k = emit_late_consts()
            for c in range(1, NCH):
                emit_chunk(c)

            # ---------------- attention (software-pipelined blocks) ---------
            PTS, OPS, SPS, RBS, ORL = {}, {}, {}, {}, {}
            ONM, RCS, OBS = {}, {}, {}

            def emit_qk(b):
                pts = []
                for kt in range(NKT):
                    pe = ps.tile([128, BL], f32, tag="e", bufs=3,
                                 name=f"e{b}_{kt}")
                    for ct in range(2):
                        mm(pe[:],
                           k_sb[ct][:, b * BL + kt * 128:
                                    b * BL + (kt + 1) * 128],
                           q_sb[ct][:, HALF + b * BL:HALF + (b + 1) * BL],
                           ct == 0, ct == 1)
                    pt = sbs.tile([128, BL], rdt, tag="pt", bufs=6,
                                  name=f"pt{b}_{kt}")
                    nc.scalar.activation(
                        pt[:], pe[:], AF.Exp,
                        bias=ab[:, b * NKT + kt:b * NKT + kt + 1],
                        scale=1.0 / 16.0)
                    pts.append(pt)
                PTS[b] = pts

            def emit_av(b):
                o0 = ps.tile([128, BL], f32, tag="o0", bufs=1, name=f"o0_{b}")
                o1 = ps.tile([128, BL], f32, tag="o1", bufs=1, name=f"o1_{b}")
                sp = ps.tile([1, BL], f32, tag="s", bufs=1, name=f"s{b}")
                for kt in range(NKT):
                    vt = vts[b * 4 + kt]
                    pt = PTS[b][kt]
                    mm(o0[:], vt[:, 0:128], pt[:], kt == 0, kt == NKT - 1)
                    mm(o1[:], vt[:, 128:256], pt[:], kt == 0, kt == NKT - 1)
                    mm(sp[:], ones_k[:], pt[:], kt == 0, kt == NKT - 1)
                OPS[b] = (o0, o1)
                SPS[b] = sp

            def emit_finA(b):
                rc = sbs.tile([1, BL], f32, tag="rc", bufs=2, name=f"rc{b}")
                nc.vector.reciprocal_approx_fast(rc[:], SPS[b][:])
                RCS[b] = rc
                rb = sbs.tile([128, BL], f32, tag="rbs", bufs=2, name=f"rb{b}")
                nc.gpsimd.partition_broadcast(rb[:], rc[:])
                RBS[b] = rb

            def emit_normrelu(b):
                orl = []
                for m in range(2):
                    on = sbs.tile([128, BL], f32, tag=f"on{m}", bufs=2,
                                  name=f"on{b}_{m}")
                    nc.vector.tensor_mul(on[:], OPS[b][m][:], RBS[b][:])
                    rl = sbs.tile([128, BL], rdt, tag=f"rl{m}", bufs=2,
                                  name=f"rl{b}_{m}")
                    nc.scalar.activation(rl[:], on[:], AF.Relu,
                                         bias=bv[:, m, :], scale=1.0)
                    orl.append(rl)
                    ONM.setdefault(b, []).append(on)
                ORL[b] = orl

            def emit_outproj(b):
                ob = sbs.tile([128, 4, BL], f32, tag="ob", bufs=1,
                              name=f"ob{b}")
                for v in range(4):
                    po = ps.tile([128, BL], f32, tag="pp", bufs=2,
                                 name=f"po{b}_{v}")
                    for m in range(2):
                        mm(po[:], wo[:, m, v * 128:(v + 1) * 128],
                           ORL[b][m][:], m == 0, m == 1)
                    nc.vector.tensor_scalar_add(ob[:, v, :], po[:],
                                                bo[:, v, :])
                OBS[b] = ob
                for v in range(4):
                    dma(out=out_r[:, v:v + 1, b * BL:(b + 1) * BL],
                        in_=ob[:, v:v + 1, :])

            for step in range(NBLK + 1):
                if 1 <= step <= NBLK:
                    emit_normrelu(step - 1)
                if step < NBLK:
                    emit_qk(step)
                    emit_av(step)
                if 1 <= step <= NBLK:
                    emit_outproj(step - 1)
                if step < NBLK:
                    emit_finA(step)

    nc.compile()
    return nc


def get_nc():
    key = MM_DTYPE
    if key not in _NC_CACHE:
        _NC_CACHE[key] = _build_nc()
    return _NC_CACHE[key]


def make_core_inputs(inputs):
    """Split full inputs into 8 per-core input maps."""
    x1 = np.ascontiguousarray(np.asarray(inputs["x1"], dtype=np.float32))
    mask = np.asarray(inputs["mask"], dtype=np.float32)
    wq_t = np.ascontiguousarray(np.asarray(inputs["Wq"], np.float32).T)
    wk_t = np.ascontiguousarray(np.asarray(inputs["Wk"], np.float32).T)
    wv_t = np.ascontiguousarray(np.asarray(inputs["Wv"], np.float32).T)
    wo_t = np.ascontiguousarray(np.asarray(inputs["Wo"], np.float32).T)
    bq = np.asarray(inputs["bq"], np.float32).reshape(C, 1)
    bk = np.asarray(inputs["bk"], np.float32).reshape(C, 1)
    bv = np.asarray(inputs["bv"], np.float32).reshape(C, 1)
    bo = np.asarray(inputs["bo"], np.float32).reshape(VD, 1)

    # padded log-mask (the reference pads mask with zeros, then adds
    # log(mask + 1e-6) to the energies)
    mp = np.pad(mask[:, 0, :], ((0, 0), (HALF, HALF)))
    lb = np.log(mp + np.float32(1e-6)).astype(np.float32)  # [B, L + 2*HALF]

    in_maps = []
    for core in range(NCORES):
        b, h = divmod(core, 2)
        s = h * LCH
        xe = np.zeros((CIN, LEXT), np.float32)
        lo, hi = s - HALF, s + LCH + HALF
        slo, shi = max(lo, 0), min(hi, L)
        xe[:, slo - lo:slo - lo + (shi - slo)] = x1[b, :, slo:shi]
        ab = np.empty((128, NBLK * NKT), np.float32)
        for blk in range(NBLK):
            w = lb[b, s + blk * BL:s + blk * BL + WS]
            ab[:, blk * NKT:(blk + 1) * NKT] = w.reshape(NKT, 128).T
        in_maps.append({
            "x": xe, "wq_t": wq_t, "wk_t": wk_t, "wv_t": wv_t, "wo_t": wo_t,
            "bq": bq, "bk": bk, "bv": bv, "bo": bo, "abias": ab,
        })
    return in_maps


def assemble_output(results):
    out = np.empty((B, VD, L), np.float32)
    for core in range(NCORES):
        b, h = divmod(core, 2)
        out[b, :, h * LCH:(h + 1) * LCH] = results[core]["out"]
    return out


LAST_RESULT = None


def kernel(**inputs):
    global LAST_RESULT
    from concourse.bass_utils import run_bass_kernel_spmd

    nc = get_nc()
    in_maps = make_core_inputs(inputs)
    res = run_bass_kernel_spmd(nc, in_maps, list(range(NCORES)))
    LAST_RESULT = res
    return assemble_output(res.results)


# revision 16
# speedup vs baseline: 1.0445x; 1.0445x over previous
"""Trainium2 Bass kernel for nn_AttLayer (sliding-block attention encoder layer).

Sharding: 8 cores = 4 batches x 2 sequence halves (4096 frames each).
Each core gets its x1 slice with a 256-frame halo on both sides (zero-padded at
sequence edges), computes q/k/v projections, 8 blocks of windowed attention
(block 512, window 1024), relu + output projection locally. No collectives.

Device layout choices:
  - q, k stored [c=256(2 ptiles), Lext=4608] in SBUF.
  - v stored TRANSPOSED [Lext(36 ptiles), c3=256]  (computed directly as
    x^T @ Wv^T so no on-chip transpose is ever needed).
  - energy computed transposed: eT[k, q] = sum_c k[c,k] q[c,q]  -> the softmax
    log-mask bias lands on the partition dim, a perfect fit for the ACT
    engine's per-partition bias operand:  P = Exp(eT/16 + bias).
  - no max-subtraction in softmax (energies are O(10), exp is safe in fp32).
  - row sums s[q] via ones-vector matmul (M=1) accumulated with AV.
  - 1/s broadcast across partitions via a K=1 ones matmul.
  - biases: bq/bk added at PSUM evacuation (per-partition); bv folded into the
    ReLU's per-partition bias AFTER normalization (exact: relu(x/s + bv));
    bo added at final evacuation.
"""

import numpy as np

# problem constants (self-contained; must match the harness reference)
B, CIN, L = 4, 512, 8192
C, VD = 256, 512
BL, HALF = 512, 256
NCORES = 8
LCH = L // 2            # 4096 frames per core
LEXT = LCH + 2 * HALF   # 4608 with halo
NBLK = LCH // BL        # 8 local blocks
WS = BL + 2 * HALF      # 1024 window
NKT = WS // 128         # 8 k-tiles per window
NCH = LEXT // BL        # 9 x chunks
NVT = LEXT // 128       # 36 v^T partition tiles

MM_DTYPE = "float32r"   # matmul compute dtype: "float32r" (fast) or "float32"

_NC_CACHE = {}


def _build_nc():
    import concourse.bacc as bacc
    import concourse.mybir as mybir
    import concourse.tile as tile
    from contextlib import ExitStack

    f32 = mybir.dt.float32
    rdt = getattr(mybir.dt, MM_DTYPE)
    AF = mybir.ActivationFunctionType

    nc = bacc.Bacc("TRN2", target_bir_lowering=False, debug=False,
                   num_devices=NCORES)

    x_d = nc.dram_tensor("x", [CIN, LEXT], rdt, kind="ExternalInput").ap()
    wq_d = nc.dram_tensor("wq_t", [CIN, C], rdt, kind="ExternalInput").ap()
    wk_d = nc.dram_tensor("wk_t", [CIN, C], rdt, kind="ExternalInput").ap()
    wv_d = nc.dram_tensor("wv_t", [CIN, C], rdt, kind="ExternalInput").ap()
    wo_d = nc.dram_tensor("wo_t", [C, VD], rdt, kind="ExternalInput").ap()
    bq_d = nc.dram_tensor("bq", [C, 1], f32, kind="ExternalInput").ap()
    bk_d = nc.dram_tensor("bk", [C, 1], f32, kind="ExternalInput").ap()
    bv_d = nc.dram_tensor("bv", [C, 1], f32, kind="ExternalInput").ap()
    bo_d = nc.dram_tensor("bo", [VD, 1], f32, kind="ExternalInput").ap()
    ab_d = nc.dram_tensor("abias", [128, NBLK * NKT], f32,
                          kind="ExternalInput").ap()
    out_d = nc.dram_tensor("out", [VD, LCH], f32, kind="ExternalOutput").ap()

    x_r = x_d.rearrange("(r p) l -> p r l", p=128)      # [128, 4, 4608]
    wq_r = wq_d.rearrange("(r p) c -> p r c", p=128)    # [128, 4, 256]
    wk_r = wk_d.rearrange("(r p) c -> p r c", p=128)
    wv_r = wv_d.rearrange("(r p) c -> p r c", p=128)
    wo_r = wo_d.rearrange("(m p) v -> p m v", p=128)    # [128, 2, 512]
    bq_r = bq_d.rearrange("(m p) o -> p m o", p=128)    # [128, 2, 1]
    bk_r = bk_d.rearrange("(m p) o -> p m o", p=128)
    bv_r = bv_d.rearrange("(m p) o -> p m o", p=128)
    bo_r = bo_d.rearrange("(v p) o -> p v o", p=128)    # [128, 4, 1]
    out_r = out_d.rearrange("(v p) l -> p v l", p=128)  # [128, 4, 4096]

    with tile.TileContext(nc) as tc:
        with ExitStack() as ctx:
            ctx.enter_context(nc.allow_low_precision(
                reason="float32r matmul pipeline; fp32 PSUM accumulation"))
            sbc = ctx.enter_context(tc.tile_pool(name="sbc", bufs=1))  # constants
            sbp = ctx.enter_context(tc.tile_pool(name="sbp", bufs=1))  # persistent
            sbs = ctx.enter_context(tc.tile_pool(name="sbs", bufs=1))  # streaming
            ps = ctx.enter_context(tc.tile_pool(name="ps", bufs=1, space="PSUM"))

            dma = nc.sync.dma_start

            wq = sbc.tile([128, 4, C], rdt, tag="wq", name="wq")
            wk = sbc.tile([128, 4, C], rdt, tag="wk", name="wk")
            wv = sbc.tile([128, 4, C], rdt, tag="wv", name="wv")
            # chunk-0 x tile is hoisted so its DMAs interleave with wq's:
            # the Sync engine issues one DMA per ~650ns, so issue order is
            # the critical path to the first matmul.
            xt0 = sbs.tile([128, 4, BL], rdt, tag="x", bufs=2, name="xt0")
            xt1 = sbs.tile([128, 4, BL], rdt, tag="x", bufs=2, name="xt1")
            bq = sbc.tile([128, 2, 1], f32, tag="bq", name="bq")
            bk = sbc.tile([128, 2, 1], f32, tag="bk", name="bk")
            dma(out=wq[:, 0:1, :], in_=wq_r[:, 0:1, :])
            dma(out=xt0[:, 0:1, :], in_=x_r[:, 0:1, 0:BL])
            dma(out=bq[:], in_=bq_r)
            dma(out=bk[:], in_=bk_r)
            for r in range(1, 4):
                dma(out=wq[:, r:r + 1, :], in_=wq_r[:, r:r + 1, :])
                dma(out=xt0[:, r:r + 1, :], in_=x_r[:, r:r + 1, 0:BL])
            xt2 = sbs.tile([128, 4, BL], rdt, tag="x", bufs=2, name="xt2")
            for r in range(4):
                dma(out=wk[:, r:r + 1, :], in_=wk_r[:, r:r + 1, :])
                dma(out=wv[:, r:r + 1, :], in_=wv_r[:, r:r + 1, :])
                dma(out=xt1[:, r:r + 1, :], in_=x_r[:, r:r + 1, BL:2 * BL])
                dma(out=xt2[:, r:r + 1, :], in_=x_r[:, r:r + 1, 2 * BL:3 * BL])

            # PE warmup: ~30 dependency-free bf16 matmuls during the DMA head
            # so the HAM clock-gate reaches 8/8 before real work arrives.
            bf16 = mybir.dt.bfloat16
            wrm = sbc.tile([128, BL], bf16, tag="wrm", name="wrm")
            nc.vector.memset(wrm[:], 0.0)
            wps = ps.tile([128, BL], f32, tag="pp", bufs=2, name="wps")
            for i in range(14):
                nc.tensor.matmul(wps[:], wrm[:, 0:128], wrm[:], start=True,
                                 stop=True)

            def emit_late_consts():
                wo = sbc.tile([128, 2, VD], rdt, tag="wo", name="wo")
                dma(out=wo[:], in_=wo_r)
                bv = sbc.tile([128, 2, 1], f32, tag="bv", name="bv")
                dma(out=bv[:], in_=bv_r)
                bo = sbc.tile([128, 4, 1], f32, tag="bo", name="bo")
                dma(out=bo[:], in_=bo_r)
                ab = sbc.tile([128, NBLK * NKT], f32, tag="ab", name="ab")
                dma(out=ab[:], in_=ab_d)
                ones_f = sbc.tile([128, 1], f32, tag="ones_f", name="ones_f")
                nc.vector.memset(ones_f[:], 1.0)
                ones_k = sbc.tile([128, 1], rdt, tag="ones_k", name="ones_k")
                nc.vector.tensor_copy(ones_k[:], ones_f[:])
                return wo, bv, bo, ab, ones_k

            q_sb = [sbp.tile([128, LEXT], rdt, tag=f"qsb{i}", name=f"qsb{i}")
                    for i in range(2)]
            k_sb = [sbp.tile([128, LEXT], rdt, tag=f"ksb{i}", name=f"ksb{i}")
                    for i in range(2)]
            vts = [sbp.tile([128, C], rdt, tag=f"vt{i}", name=f"vt{i}")
                   for i in range(NVT)]

            def mm(out_ap, lhsT, rhs, start, stop):
                nc.tensor.matmul(out_ap, lhsT, rhs, start=start, stop=stop)

            # ---------------- projections (streamed over 9 x-chunks) --------
            def emit_chunk(c):
                if c == 0:
                    xt = xt0
                elif c == 1:
                    xt = xt1
                elif c == 2:
                    xt = xt2
                else:
                    xt = sbs.tile([128, 4, BL], rdt, tag="x", bufs=2,
                                  name=f"xt{c}")
                    for r in range(4):
                        dma(out=xt[:, r:r + 1, :],
                            in_=x_r[:, r:r + 1, c * BL:(c + 1) * BL])
                # q is only needed on extended cols [HALF, LEXT-HALF)
                qlo = max(c * BL, HALF) - c * BL
                qhi = min((c + 1) * BL, LEXT - HALF) - c * BL
                for o in range(2):
                    pq = ps.tile([128, BL], f32, tag="pp", bufs=2,
                                 name=f"pq{c}_{o}")
                    for r in range(4):
                        mm(pq[:, 0:qhi - qlo],
                           wq[:, r, o * 128:(o + 1) * 128],
                           xt[:, r, qlo:qhi], r == 0, r == 3)
                    nc.vector.tensor_scalar_add(
                        q_sb[o][:, c * BL + qlo:c * BL + qhi],
                        pq[:, 0:qhi - qlo], bq[:, o, :])
                for o in range(2):
                    pk = ps.tile([128, BL], f32, tag="pp", bufs=2,
                                 name=f"pk{c}_{o}")
                    for r in range(4):
                        mm(pk[:], wk[:, r, o * 128:(o + 1) * 128], xt[:, r, :],
                           r == 0, r == 3)
                    nc.vector.tensor_scalar_add(
                        k_sb[o][:, c * BL:(c + 1) * BL], pk[:], bk[:, o, :])
                for lt in range(4):
                    pv = ps.tile([128, C], f32, tag="pp", bufs=2,
                                 name=f"pv{c}_{lt}")
                    for r in range(4):
                        mm(pv[:], xt[:, r, lt * 128:(lt + 1) * 128],
                           wv[:, r, :], r == 0, r == 3)
                    nc.vector.tensor_copy(vts[c * 4 + lt][:], pv[:])

            emit_chunk(0)
            wo, bv, bo, ab, ones_k = emit_late_consts()
            for c in range(1, NCH):
                emit_chunk(c)

            # ---------------- attention (software-pipelined blocks) ---------
            PTS, OPS, SPS, RBS, ORL = {}, {}, {}, {}, {}
            ONM, RCS, OBS = {}, {}, {}

            def emit_qk(b):
                pts = []
                for kt in range(NKT):
                    pe = ps.tile([128, BL], f32, tag="e", bufs=3,
                                 name=f"e{b}_{kt}")
                    for ct in range(2):
                        mm(pe[:],
                           k_sb[ct][:, b * BL + kt * 128:
                                    b * BL + (kt + 1) * 128],
                           q_sb[ct][:, HALF + b * BL:HALF + (b + 1) * BL],
                           ct == 0, ct == 1)
                    pt = sbs.tile([128, BL], rdt, tag="pt", bufs=6,
                                  name=f"pt{b}_{kt}")
                    nc.scalar.activation(
                        pt[:], pe[:], AF.Exp,
                        bias=ab[:, b * NKT + kt:b * NKT + kt + 1],
                        scale=1.0 / 16.0)
                    pts.append(pt)
                PTS[b] = pts

            def emit_av(b):
                o0 = ps.tile([128, BL], f32, tag="o0", bufs=1, name=f"o0_{b}")
                o1 = ps.tile([128, BL], f32, tag="o1", bufs=1, name=f"o1_{b}")
                sp = ps.tile([1, BL], f32, tag="s", bufs=1, name=f"s{b}")
                sa = sbs.tile([128, BL], f32, tag="sa", bufs=2,
                              name=f"sa{b}")
                for kt in range(NKT):
                    vt = vts[b * 4 + kt]
                    pt = PTS[b][kt]
                    mm(o0[:], vt[:, 0:128], pt[:], kt == 0, kt == NKT - 1)
                    mm(o1[:], vt[:, 128:256], pt[:], kt == 0, kt == NKT - 1)
                    # row-sum accumulates on DVE in f32 (exact), off the PE
                    if kt == 1:
                        nc.vector.tensor_add(sa[:], PTS[b][0][:], pt[:])
                    elif kt > 1:
                        nc.vector.tensor_add(sa[:], sa[:], pt[:])
                sar = sbs.tile([128, BL], rdt, tag="sar", bufs=2,
                               name=f"sar{b}")
                nc.vector.tensor_copy(sar[:], sa[:])
                mm(sp[:], ones_k[:], sar[:], True, True)
                OPS[b] = (o0, o1)
                SPS[b] = sp

            def emit_finA(b):
                rc = sbs.tile([1, BL], f32, tag="rc", bufs=2, name=f"rc{b}")
                nc.vector.reciprocal_approx_fast(rc[:], SPS[b][:])
                RCS[b] = rc
                rb = sbs.tile([128, BL], f32, tag="rbs", bufs=2, name=f"rb{b}")
                nc.gpsimd.partition_broadcast(rb[:], rc[:])
                RBS[b] = rb

            def emit_normrelu(b):
                orl = []
                for m in range(2):
                    on = sbs.tile([128, BL], f32, tag=f"on{m}", bufs=2,
                                  name=f"on{b}_{m}")
                    nc.vector.tensor_mul(on[:], OPS[b][m][:], RBS[b][:])
                    rl = sbs.tile([128, BL], rdt, tag=f"rl{m}", bufs=2,
                                  name=f"rl{b}_{m}")
                    nc.scalar.activation(rl[:], on[:], AF.Relu,
                                         bias=bv[:, m, :], scale=1.0)
                    orl.append(rl)
                    ONM.setdefault(b, []).append(on)
                ORL[b] = orl

            def emit_outproj(b):
                ob = sbs.tile([128, 4, BL], f32, tag="ob", bufs=1,
                              name=f"ob{b}")
                for v in range(4):
                    po = ps.tile([128, BL], f32, tag="pp", bufs=2,
                                 name=f"po{b}_{v}")
                    for m in range(2):
                        mm(po[:], wo[:, m, v * 128:(v + 1) * 128],
                           ORL[b][m][:], m == 0, m == 1)
                    nc.vector.tensor_scalar_add(ob[:, v, :], po[:],
                                                bo[:, v, :])
                OBS[b] = ob
                for v in range(4):
                    dma(out=out_r[:, v:v + 1, b * BL:(b + 1) * BL],
                        in_=ob[:, v:v + 1, :])

            for step in range(NBLK + 1):
                if 1 <= step <= NBLK:
                    emit_normrelu(step - 1)
                if step < NBLK:
                    emit_qk(step)
                    emit_av(step)
                if 1 <= step <= NBLK:
                    emit_outproj(step - 1)
                if step < NBLK:
                    emit_finA(step)

    nc.compile()
    return nc


def get_nc():
    key = MM_DTYPE
    if key not in _NC_CACHE:
        _NC_CACHE[key] = _build_nc()
    return _NC_CACHE[key]


def make_core_inputs(inputs):
    """Split full inputs into 8 per-core input maps."""
    x1 = np.ascontiguousarray(np.asarray(inputs["x1"], dtype=np.float32))
    mask = np.asarray(inputs["mask"], dtype=np.float32)
    wq_t = np.ascontiguousarray(np.asarray(inputs["Wq"], np.float32).T)
    wk_t = np.ascontiguousarray(np.asarray(inputs["Wk"], np.float32).T)
    wv_t = np.ascontiguousarray(np.asarray(inputs["Wv"], np.float32).T)
    wo_t = np.ascontiguousarray(np.asarray(inputs["Wo"], np.float32).T)
    bq = np.asarray(inputs["bq"], np.float32).reshape(C, 1)
    bk = np.asarray(inputs["bk"], np.float32).reshape(C, 1)
    bv = np.asarray(inputs["bv"], np.float32).reshape(C, 1)
    bo = np.asarray(inputs["bo"], np.float32).reshape(VD, 1)

    # padded log-mask (the reference pads mask with zeros, then adds
    # log(mask + 1e-6) to the energies)
    mp = np.pad(mask[:, 0, :], ((0, 0), (HALF, HALF)))
    lb = np.log(mp + np.float32(1e-6)).astype(np.float32)  # [B, L + 2*HALF]

    in_maps = []
    for core in range(NCORES):
        b, h = divmod(core, 2)
        s = h * LCH
        xe = np.zeros((CIN, LEXT), np.float32)
        lo, hi = s - HALF, s + LCH + HALF
        slo, shi = max(lo, 0), min(hi, L)
        xe[:, slo - lo:slo - lo + (shi - slo)] = x1[b, :, slo:shi]
        ab = np.empty((128, NBLK * NKT), np.float32)
        for blk in range(NBLK):
            w = lb[b, s + blk * BL:s + blk * BL + WS]
            ab[:, blk * NKT:(blk + 1) * NKT] = w.reshape(NKT, 128).T
        in_maps.append({
            "x": xe, "wq_t": wq_t, "wk_t": wk_t, "wv_t": wv_t, "wo_t": wo_t,
            "bq": bq, "bk": bk, "bv": bv, "bo": bo, "abias": ab,
        })
    return in_maps


def assemble_output(results):
    out = np.empty((B, VD, L), np.float32)
    for core in range(NCORES):
        b, h = divmod(core, 2)
        out[b, :, h * LCH:(h + 1) * LCH] = results[core]["out"]
    return out


LAST_RESULT = None


def kernel(**inputs):
    global LAST_RESULT
    from concourse.bass_utils import run_bass_kernel_spmd

    nc = get_nc()
    in_maps = make_core_inputs(inputs)
    res = run_bass_kernel_spmd(nc, in_maps, list(range(NCORES)))
    LAST_RESULT = res
    return assemble_output(res.results)
